# revision 8
# baseline (speedup 1.0000x reference)
"""Fused multi-head attention (B=2, T=2048, D=2048, H=16) on 8 trn2 NeuronCores.

Sharding: core c handles batch b=c//4 and heads [4g, 4g+4), g=c%4 (tensor
parallel over heads x data parallel over batch). Each core computes its
4 heads' contribution to out[b] = attn(x[b]) @ Wo^T; the host sums the 4
partials per batch.

v4: the attention phase is co-limited by the PE and the DVE, so the DVE
work is split across engines: softmax-denominator accumulation plane 0
stays on the DVE while plane 1 runs on the (otherwise idle) GPSIMD, and
the P3 psum->SBUF copies alternate ACT/DVE. Diagonal mask blocks are
processed as per-plane query windows (512/384/256/128 columns) so the
causal triangle costs only a [128,2,128] mask multiply and ~25% fewer
S/ctx matmul columns. x^T is staged block-major (4KB DMA runs), chunk 0
of the attention rides interleaved between the second-half q/k
projection units to hide its exp warmup.

Device algorithm (per core, E=512 features = 4 heads):
  P1  v = x @ Wv_s^T [T, E]; qT/kT = (W_s) @ x^T [E, T] for cols 0:1024;
      cols 1024:2048 interleaved with P2 chunk 0.
  P2  per i-chunk (512 q), pseudo-pair of key tiles: S^T = kT^T-contract
      @ qT -> exp (ACT) -> *triangle (diag only, DVE) -> zt plane adds
      (DVE/GPSIMD) -> ctx^T += v^T @ P^T; L = ones^T @ zt (2 chained);
      ctx^T *= recip(L).  PE slack filled with P3 units of chunk ic-1.
  P3  out[t, d] = sum_e ctx^T[e, t] * WoT[e, d] -> DRAM (fp16)
"""

import numpy as np

import concourse.bass as bass
import concourse.mybir as mybir
import concourse.tile as tile
from concourse import bacc
from concourse.bass_utils import run_bass_kernel_spmd

F32 = mybir.dt.float32
F16 = mybir.dt.float16
EXP = mybir.ActivationFunctionType.Exp

B, T, D, H = 2, 2048, 2048, 16
DH = D // H          # 128
E = 512              # features per core (4 heads)
HPC = 4              # heads per core
NT = T // 128        # 16 token tiles
ND = D // 128        # 16 model-dim tiles
NE = E // 128        # 4 e-tiles per core
NI = T // 512        # 4 i-chunks (query chunks)
NJ = NT              # 16 j-tiles (key tiles)

_NC_CACHE = {}

# per-(jt, ic) mask-block class: 0 = fully masked (skip), 1 = unmasked
# (skip the mask multiply), 2 = mixed (multiply by exp(mask) elementwise)
SKIP, NOMULT, MIXED = 0, 1, 2


def _build(cls_key, causal):
    cls = np.asarray(cls_key, dtype=np.int64).reshape(NJ, NI)
    nc = bacc.Bacc(None, target_bir_lowering=False, debug=False)
    # x^T staged block-major: [128 dpart, ttile, dtile, 128 tcol] so every
    # per-token-block DMA is a 4KB-contiguous run per partition
    xt = nc.declare_dram_parameter("xt", [128, NT, ND, 128], F16, isOutput=False)
    wq = nc.declare_dram_parameter("wq", [128, ND, E], F16, isOutput=False)
    wk = nc.declare_dram_parameter("wk", [128, ND, E], F16, isOutput=False)
    wv = nc.declare_dram_parameter("wv", [128, ND, E], F16, isOutput=False)
    wo = nc.declare_dram_parameter("wo", [128, NE, D], F16, isOutput=False)
    if causal:
        # duplicated upper-triangle pattern for the diagonal mask windows
        trip = nc.declare_dram_parameter("trip", [128, 2, 128], F16, isOutput=False)
    else:
        em = nc.declare_dram_parameter("em", [T, T], F16, isOutput=False)
    out = nc.declare_dram_parameter("out", [T, D], F16, isOutput=True)

    with tile.TileContext(nc) as tc:
        # ---- long-lived residents --------------------------------------
        pool_res = tc.alloc_tile_pool(name="res", bufs=1)
        ctx = [pool_res.tile([128, T], F16, name=f"ctx{m}") for m in range(NE)]
        v_sb = pool_res.tile([128, NT, E], F16)
        wo_sb = pool_res.tile([128, NE, D], F16)
        ones_sb = pool_res.tile([128, 128], F16)
        scratch = pool_res.tile([1, 8], F16)
        if causal:
            trip_sb = pool_res.tile([128, 2, 128], F16)

        pool_qk = tc.alloc_tile_pool(name="res_qk", bufs=1)
        qT = [pool_qk.tile([128, T], F16, name=f"qT{m}") for m in range(NE)]
        kT = [pool_qk.tile([128, T], F16, name=f"kT{m}") for m in range(NE)]

        # One set of PSUM pools for every phase (no pool transitions):
        # s = S-tile pairs (2 banks x 2 bufs), c = everything else
        # (v-pass / q/k units / ctx accumulators / P3 / L row-sums).
        ps_s_pool = tc.alloc_tile_pool(name="pss", bufs=2, space="PSUM")
        ps_c_pool = tc.alloc_tile_pool(name="psc", bufs=4, space="PSUM")

        # pool_b outlives pool_a (stack order: a on top so its release
        # frees space for the P2 SBUF pools while b stays live for tch1)
        pool_b = tc.alloc_tile_pool(name="p1b", bufs=1)
        xt_hi = pool_b.tile([128, 8, ND, 128], F16)
        wq_sb = pool_b.tile([128, ND, E], F16)
        wk_sb = pool_b.tile([128, ND, E], F16)
        pool_a = tc.alloc_tile_pool(name="p1a", bufs=1)
        xt_lo = pool_a.tile([128, 8, ND, 128], F16)
        wv_sb = pool_a.tile([128, ND, E], F16)

        def xt_blk(tb):
            return (xt_lo[:, tb] if tb < 8 else xt_hi[:, tb - 8])

        # ---- DMA schedule: wv chunks + first token blocks first so the
        # v-pass first matmul gates on ~640KB; the rest streams under the
        # v-pass compute.
        nc.vector.memset(ones_sb, 1.0)
        # warm the ACT exp table set without waiting on any DMA
        nc.scalar.activation(scratch[0:1, 0:1], ones_sb[0:1, 0:1], EXP)

        nc.sync.dma_start(out=wv_sb[:, 0:4, :], in_=wv.ap()[:, 0:4, :])
        for qq in range(4):
            nc.sync.dma_start(out=xt_lo[:, 0, 4 * qq:4 * qq + 4, :],
                              in_=xt.ap()[:, 0, 4 * qq:4 * qq + 4, :])
        for qq in range(1, 4):
            nc.sync.dma_start(out=wv_sb[:, 4 * qq:4 * qq + 4, :],
                              in_=wv.ap()[:, 4 * qq:4 * qq + 4, :])
        for tb in range(1, 4):
            nc.sync.dma_start(out=xt_lo[:, tb], in_=xt.ap()[:, tb])
        for tb in range(4, 8, 2):
            nc.sync.dma_start(out=xt_lo[:, tb:tb + 2], in_=xt.ap()[:, tb:tb + 2])
        for tb in range(8, 16, 2):
            nc.sync.dma_start(out=xt_hi[:, tb - 8:tb - 6],
                              in_=xt.ap()[:, tb:tb + 2])
        nc.sync.dma_start(out=wq_sb, in_=wq.ap())
        nc.sync.dma_start(out=wk_sb, in_=wk.ap())
        if causal:
            nc.sync.dma_start(out=trip_sb, in_=trip.ap())
        nc.sync.dma_start(out=wo_sb, in_=wo.ap())

        scope_p1 = nc.named_scope("P1_qkv"); scope_p1.__enter__()
        # ---- P1: v (token-major) first, then q/k cols 0:1024 ------------
        for tbg in range(NT):
            ps = ps_c_pool.tile([128, 512], F32, name="ps_v", tag="c")
            for dt in range(ND):
                nc.tensor.matmul(
                    ps, xt_blk(tbg)[:, dt, :],
                    wv_sb[:, dt, :], start=(dt == 0), stop=(dt == ND - 1))
            if tbg % 2 == 0:
                nc.vector.tensor_copy(v_sb[:, tbg, :], ps)
            else:
                nc.scalar.copy(v_sb[:, tbg, :], ps)

        def qk_unit(tch, ti, m, half):
            w_sb = wq_sb if ti == 0 else wk_sb
            dst = qT if ti == 0 else kT
            tb0 = tch * 8 + half * 4
            tsl = slice(tch * 1024 + half * 512, tch * 1024 + (half + 1) * 512)
            ps = ps_c_pool.tile([128, 512], F32, name="ps_qk", tag="c")
            for dt in range(ND):
                src = xt_lo if tb0 < 8 else xt_hi
                b0 = tb0 if tb0 < 8 else tb0 - 8
                nc.tensor.matmul(ps, w_sb[:, dt, m * 128:(m + 1) * 128],
                                 src[:, b0:b0 + 4, dt, :],
                                 start=(dt == 0), stop=(dt == ND - 1))
            if (m + ti + half) % 2 == 0:
                nc.scalar.copy(dst[m][:, tsl], ps)
            else:
                nc.vector.tensor_copy(dst[m][:, tsl], ps)

        for ti in range(2):
            for m in range(NE):
                for half in range(2):
                    qk_unit(0, ti, m, half)
        pool_a.release()
        scope_p1.__exit__(None, None, None)

        scope_p2 = nc.named_scope("P2_attn"); scope_p2.__enter__()
        # ---- P2 SBUF pools (fit in the space pool_a released) -----------
        p_pt = tc.alloc_tile_pool(name="p2pt", bufs=6)
        p_em = tc.alloc_tile_pool(name="p2em", bufs=3)
        p_bs = tc.alloc_tile_pool(name="p2bs", bufs=3)
        p_z = tc.alloc_tile_pool(name="p2z", bufs=3)
        p_ot = tc.alloc_tile_pool(name="p2ot", bufs=6)

        def p3_unit(tt, nch):
            tsl = slice(tt * 128, (tt + 1) * 128)
            ps_o = ps_c_pool.tile([128, 512], F32, name="ps_o", tag="c")
            for et in range(NE):
                nc.tensor.matmul(
                    ps_o, ctx[et][:, tsl],
                    wo_sb[:, et, nch * 512:(nch + 1) * 512],
                    start=(et == 0), stop=(et == NE - 1))
            ot = p_ot.tile([128, 512], F16, name="ot")
            if (tt + nch) % 2 == 0:
                nc.scalar.copy(ot, ps_o)
            else:
                nc.vector.tensor_copy(ot, ps_o)
            nc.sync.dma_start(
                out=out.ap()[tsl, nch * 512:(nch + 1) * 512], in_=ot)

        def attn_chunk(ic, filler):
            """Emit attention for i-chunk ic; call filler() in PE slack.

            Key tiles are processed as pseudo-pairs [(jt, qo, qn) x 2]:
            plane j covers query columns [qo, qo+qn).  Causal diagonal
            tiles get shrinking query windows; their triangle lives in
            the first 128 columns of each window (trip mask).
            """
            isl = slice(ic * 512, (ic + 1) * 512)
            surv = [jt for jt in range(NJ) if cls[jt, ic] != SKIP]
            assert surv, f"i-chunk {ic}: every key block masked"
            first, last = surv[0], surv[-1]
            if causal:
                nodiag = [jt for jt in surv if cls[jt, ic] != MIXED]
                diag = [jt for jt in surv if cls[jt, ic] == MIXED]
                prs = [[(jt, 0, 512) for jt in nodiag[i:i + 2]]
                       for i in range(0, len(nodiag), 2)]
                for kk in range(len(diag) // 2):
                    prs.append([(diag[2 * kk + j], 128 * (2 * kk + j),
                                 512 - 128 * (2 * kk + j)) for j in range(2)])
            else:
                prs = [[(jt, 0, 512) for jt in surv[i:i + 2]]
                       for i in range(0, len(surv), 2)]
            for h in range(HPC):
                cps = ps_c_pool.tile([128, 512], F32, name="ps_c", tag="c")
                zt = p_z.tile([128, 2, 512], F16, name="zt")
                zinit = [False, False]

                def flush(prev):
                    ppt, pjs = prev
                    for j, (jt, qo, qn) in enumerate(pjs):
                        nc.tensor.matmul(
                            cps[:, qo:qo + qn],
                            v_sb[:, jt, h * 128:(h + 1) * 128],
                            ppt[:, j, 0:qn], start=(jt == first),
                            stop=(jt == last), skip_group_check=True)

                prev = None
                for pjs in prs:
                    np_ = len(pjs)
                    mx = max(qn for (_, _, qn) in pjs)
                    isdiag = causal and cls[pjs[0][0], ic] == MIXED
                    ps_s = ps_s_pool.tile([128, 2, 512], F32, name="ps_s")
                    for j, (jt, qo, qn) in enumerate(pjs):
                        nc.tensor.matmul(
                            ps_s[:, j, 0:qn], kT[h][:, jt * 128:(jt + 1) * 128],
                            qT[h][:, ic * 512 + qo:ic * 512 + qo + qn],
                            start=True, stop=True)
                    pt = p_pt.tile([128, 2, 512], F16, name="pt")
                    if np_ == 2:
                        nc.scalar.activation(pt[:, :, 0:mx], ps_s[:, :, 0:mx], EXP)
                    else:
                        nc.scalar.activation(pt[:, 0, 0:mx], ps_s[:, 0, 0:mx], EXP)
                    if isdiag:
                        nc.vector.tensor_mul(
                            pt[:, 0:np_, 0:128], pt[:, 0:np_, 0:128],
                            trip_sb[:, 0:np_, :])
                    elif not causal and any(cls[jt, ic] == MIXED
                                            for (jt, _, _) in pjs):
                        emt = p_em.tile([128, 2, 512], F16, name="emt")
                        for j, (jt, _, _) in enumerate(pjs):
                            if cls[jt, ic] == MIXED:
                                nc.sync.dma_start(
                                    out=emt[:, j, :],
                                    in_=em.ap()[jt * 128:(jt + 1) * 128, isl])
                            else:
                                nc.vector.memset(emt[:, j, :], 1.0)
                        if np_ == 2:
                            nc.vector.tensor_mul(pt, pt, emt)
                        else:
                            nc.vector.tensor_mul(
                                pt[:, 0, :], pt[:, 0, :], emt[:, 0, :])
                    # softmax denominators: plane 0 on DVE, plane 1 GPSIMD
                    for j, (jt, qo, qn) in enumerate(pjs):
                        eng = nc.vector if j == 0 else nc.gpsimd
                        zsl = slice(qo, qo + qn)
                        with nc.allow_low_precision(reason="softmax denom f16"):
                            if not zinit[j]:
                                zinit[j] = True
                                if qn == 512:
                                    eng.tensor_copy(zt[:, j, :], pt[:, j, :])
                                else:
                                    eng.memset(zt[:, j, :], 0.0)
                                    eng.tensor_add(zt[:, j, zsl], zt[:, j, zsl],
                                                   pt[:, j, 0:qn])
                            else:
                                eng.tensor_add(zt[:, j, zsl], zt[:, j, zsl],
                                               pt[:, j, 0:qn])
                    if prev is not None:
                        # fill unit BEFORE the ctx flush: covers the first
                        # exp's latency at each head start
                        filler(h)
                        flush(prev)
                    prev = (pt, pjs)
                flush(prev)
                if not zinit[1]:
                    nc.gpsimd.memset(zt[:, 1, :], 0.0)
                lps = ps_c_pool.tile([128, 512], F32, name="ps_l", tag="c")
                for j in range(2):
                    nc.tensor.matmul(lps, ones_sb, zt[:, j, :],
                                     start=(j == 0), stop=(j == 1))
                bsb = p_bs.tile([128, 512], F32, name="bsb")
                nc.vector.reciprocal_approx_fast(out=bsb, in_=lps)
                nc.vector.tensor_mul(ctx[h][:, isl], cps, bsb)
                filler(h, tail=True)

        # ---- i-chunk 0 interleaved with q/k cols 1024:2048 --------------
        # chunk 0 has no P3 filler units, so its exp warmup would stall the
        # PE; instead its pair-steps ride between tch=1 projection units.
        if causal:
            units1 = [(ti, m, half)
                      for ti in range(2) for m in range(NE) for half in range(2)]

            def p1_filler(h, tail=False):
                n = len(units1) if (tail and h == HPC - 1) else 1
                for _ in range(n):
                    if units1:
                        qk_unit(1, *units1.pop(0))
            attn_chunk(0, p1_filler)
            while units1:
                qk_unit(1, *units1.pop(0))
            ic_start = 1
        else:
            for ti in range(2):
                for m in range(NE):
                    for half in range(2):
                        qk_unit(1, ti, m, half)
            ic_start = 0

        # ---- P2 main: chunks with P3 units of chunk ic-1 as filler ------
        for ic in range(ic_start, NI):
            ic_units = [(4 * (ic - 1) + tt, u)
                        for tt in range(4) for u in range(4)] if ic else []
            state = {"h": -1, "skip": 0, "inunits": 0}

            def p3_filler(h, tail=False, ic=ic, ic_units=ic_units, state=state):
                if state["h"] != h:
                    # h0's first slot stays empty: the previous chunk's last
                    # ctx normalization is still draining on the DVE
                    state["h"] = h
                    state["skip"] = 1 if h == 0 else 0
                    state["inunits"] = 0
                if tail:
                    nflush = (len(ic_units) if h == HPC - 1
                              else min(2, len(ic_units)))
                    for _ in range(nflush):
                        p3_unit(*ic_units.pop(0))
                    return
                if state["skip"]:
                    state["skip"] -= 1
                elif ic_units and state["inunits"] < 2:
                    p3_unit(*ic_units.pop(0))
                    state["inunits"] += 1
            attn_chunk(ic, p3_filler)
        scope_p2.__exit__(None, None, None)
        scope_p3 = nc.named_scope("P3_out"); scope_p3.__enter__()
        # ---- P3 tail: last token chunk ----------------------------------
        for tt in range(12, 16):
            for nch in range(NI):
                p3_unit(tt, nch)
        for p in (p_ot, p_z, p_bs, p_em, p_pt):
            p.release()
        pool_b.release()
        for p in (ps_c_pool, ps_s_pool):
            p.release()
        pool_qk.release()
        pool_res.release()
        scope_p3.__exit__(None, None, None)

    nc.compile()
    return nc


def _get_nc(cls_key, causal):
    key = (cls_key, causal)
    if key not in _NC_CACHE:
        _NC_CACHE[key] = _build(cls_key, causal)
    return _NC_CACHE[key]


def _causal_pattern(o):
    p = np.arange(128)[:, None]
    f = np.arange(512)[None, :]
    return (p + o * 128 <= f).astype(np.float16)


def kernel(x, Wq, Wk, Wv, Wo, attn_mask):
    x = np.asarray(x, dtype=np.float32)
    Wq = np.asarray(Wq, dtype=np.float32)
    Wk = np.asarray(Wk, dtype=np.float32)
    Wv = np.asarray(Wv, dtype=np.float32)
    Wo = np.asarray(Wo, dtype=np.float32)
    mask = np.asarray(attn_mask, dtype=np.float32).reshape(T, T)

    emT = np.ascontiguousarray(np.exp(mask).T)
    scale = np.float32(1.0 / np.sqrt(DH))

    blocks = emT.reshape(NJ, 128, NI, 512)
    cls = np.full((NJ, NI), MIXED, dtype=np.int64)
    for jt in range(NJ):
        for ic in range(NI):
            sub = blocks[jt, :, ic, :]
            if not sub.any():
                cls[jt, ic] = SKIP
            elif np.all(sub == 1.0):
                cls[jt, ic] = NOMULT
    cls_key = tuple(cls.flatten().tolist())

    # causal fast path: survivors are a prefix, MIXED blocks are the last 4
    # of each i-chunk and match the canonical diagonal patterns
    causal = True
    pat = [_causal_pattern(o).astype(np.float32) for o in range(4)]
    for ic in range(NI):
        surv = [jt for jt in range(NJ) if cls[jt, ic] != SKIP]
        mix = [jt for jt in range(NJ) if cls[jt, ic] == MIXED]
        if surv != list(range(4 * ic + 4)) or mix != list(range(4 * ic, 4 * ic + 4)):
            causal = False
            break
        for jt in mix:
            if not np.array_equal(blocks[jt, :, ic, :], pat[jt - 4 * ic]):
                causal = False
                break
        if not causal:
            break

    # upper-triangle [128,128] pattern, duplicated for both pair planes
    tri = (np.arange(128)[:, None] <= np.arange(128)[None, :])
    trip = np.ascontiguousarray(
        np.broadcast_to(tri.astype(np.float16), (2, 128, 128)).transpose(1, 0, 2))

    def _perm(a, ntile):
        # [ntile*128, F] -> [128, ntile, F] contiguous fp16
        f = a.shape[1]
        return np.ascontiguousarray(
            a.reshape(ntile, 128, f).transpose(1, 0, 2)).astype(np.float16)

    def _perm_blk(a):
        # x[b]: [T, D] -> xt[p, tb, nd, c] = x[tb*128+c, nd*128+p]
        r = a.reshape(NT, 128, ND, 128)
        return np.ascontiguousarray(r.transpose(3, 0, 2, 1)).astype(np.float16)

    xT = [_perm_blk(x[b]) for b in range(B)]
    emT16 = emT.astype(np.float16)

    in_maps = []
    for c in range(8):
        b, g = c // 4, c % 4
        rows = slice(E * g, E * (g + 1))
        m = {
            "xt": xT[b],
            "wq": _perm((Wq[rows, :] * scale).T, ND),
            "wk": _perm(Wk[rows, :].T, ND),
            "wv": _perm(Wv[rows, :].T, ND),
            "wo": _perm(Wo[:, rows].T, NE),
        }
        if causal:
            m["trip"] = trip
        else:
            m["em"] = emT16
        in_maps.append(m)

    global _LAST_IN_MAPS, _LAST_NC
    _LAST_IN_MAPS = in_maps
    nc = _get_nc(cls_key, causal)
    _LAST_NC = nc
    res = run_bass_kernel_spmd(nc, in_maps, list(range(8)))
    outs = [r["out"].astype(np.float32) for r in res.results]
    full = np.stack([
        outs[0] + outs[1] + outs[2] + outs[3],
        outs[4] + outs[5] + outs[6] + outs[7],
    ]).astype(np.float32)
    return full


# revision 14
# speedup vs baseline: 1.0627x; 1.0627x over previous
"""Fused multi-head attention (B=2, T=2048, D=2048, H=16) on 8 trn2 NeuronCores.

Sharding: core c handles batch b=c//4 and heads [4g, 4g+4), g=c%4 (tensor
parallel over heads x data parallel over batch). Each core computes its
4 heads' contribution to out[b] = attn(x[b]) @ Wo^T; the host sums the 4
partials per batch.

v4: the attention phase is co-limited by the PE and the DVE, so the DVE
work is split across engines: softmax-denominator accumulation plane 0
stays on the DVE while plane 1 runs on the (otherwise idle) GPSIMD, and
the P3 psum->SBUF copies alternate ACT/DVE. Diagonal mask blocks are
processed as per-plane query windows (512/384/256/128 columns) so the
causal triangle costs only a [128,2,128] mask multiply and ~25% fewer
S/ctx matmul columns. x^T is staged block-major (4KB DMA runs), chunk 0
of the attention rides interleaved between the second-half q/k
projection units to hide its exp warmup.

Device algorithm (per core, E=512 features = 4 heads):
  P1  v = x @ Wv_s^T [T, E]; qT/kT = (W_s) @ x^T [E, T] for cols 0:1024;
      cols 1024:2048 interleaved with P2 chunk 0.
  P2  per i-chunk (512 q), pseudo-pair of key tiles: S^T = kT^T-contract
      @ qT -> exp (ACT) -> *triangle (diag only, DVE) -> zt plane adds
      (DVE/GPSIMD) -> ctx^T += v^T @ P^T; L = ones^T @ zt (2 chained);
      ctx^T *= recip(L).  PE slack filled with P3 units of chunk ic-1.
  P3  out[t, d] = sum_e ctx^T[e, t] * WoT[e, d] -> DRAM (fp16)
"""

import numpy as np

import concourse.bass as bass
import concourse.mybir as mybir
import concourse.tile as tile
from concourse import bacc
from concourse.bass_utils import run_bass_kernel_spmd

F32 = mybir.dt.float32
F16 = mybir.dt.float16
EXP = mybir.ActivationFunctionType.Exp

B, T, D, H = 2, 2048, 2048, 16
DH = D // H          # 128
E = 512              # features per core (4 heads)
HPC = 4              # heads per core
NT = T // 128        # 16 token tiles
ND = D // 128        # 16 model-dim tiles
NE = E // 128        # 4 e-tiles per core
NI = T // 512        # 4 i-chunks (query chunks)
NJ = NT              # 16 j-tiles (key tiles)

_NC_CACHE = {}

# per-(jt, ic) mask-block class: 0 = fully masked (skip), 1 = unmasked
# (skip the mask multiply), 2 = mixed (multiply by exp(mask) elementwise)
SKIP, NOMULT, MIXED = 0, 1, 2


def _build(cls_key, causal):
    cls = np.asarray(cls_key, dtype=np.int64).reshape(NJ, NI)
    nc = bacc.Bacc(None, target_bir_lowering=False, debug=False)
    # x^T staged block-major: [128 dpart, ttile, dtile, 128 tcol] so every
    # per-token-block DMA is a 4KB-contiguous run per partition
    xt = nc.declare_dram_parameter("xt", [128, NT, ND, 128], F16, isOutput=False)
    wq = nc.declare_dram_parameter("wq", [128, ND, E], F16, isOutput=False)
    wk = nc.declare_dram_parameter("wk", [128, ND, E], F16, isOutput=False)
    wv = nc.declare_dram_parameter("wv", [128, ND, E], F16, isOutput=False)
    wo = nc.declare_dram_parameter("wo", [128, NE, D], F16, isOutput=False)
    if causal:
        # duplicated upper-triangle pattern for the diagonal mask windows
        trip = nc.declare_dram_parameter("trip", [128, 2, 128], F16, isOutput=False)
    else:
        em = nc.declare_dram_parameter("em", [T, T], F16, isOutput=False)
    out = nc.declare_dram_parameter("out", [T, D], F16, isOutput=True)

    with tile.TileContext(nc) as tc:
        # ---- long-lived residents --------------------------------------
        pool_res = tc.alloc_tile_pool(name="res", bufs=1)
        ctx = [pool_res.tile([128, T], F16, name=f"ctx{m}") for m in range(NE)]
        v_sb = pool_res.tile([128, NT, E], F16)
        wo_sb = pool_res.tile([128, NE, D], F16)
        ones_sb = pool_res.tile([128, 128], F16)
        scratch = pool_res.tile([1, 8], F16)
        if causal:
            trip_sb = pool_res.tile([128, 2, 128], F16)

        pool_qk = tc.alloc_tile_pool(name="res_qk", bufs=1)
        qT = [pool_qk.tile([128, T], F16, name=f"qT{m}") for m in range(NE)]
        kT = [pool_qk.tile([128, T], F16, name=f"kT{m}") for m in range(NE)]

        # One set of PSUM pools for every phase (no pool transitions):
        # s = S-tile pairs (2 banks x 2 bufs), c = everything else
        # (v-pass / q/k units / ctx accumulators / P3 / L row-sums).
        ps_s_pool = tc.alloc_tile_pool(name="pss", bufs=2, space="PSUM")
        ps_c_pool = tc.alloc_tile_pool(name="psc", bufs=3, space="PSUM")
        ps_l_pool = tc.alloc_tile_pool(name="psl", bufs=1, space="PSUM")

        # pool_b outlives pool_a (stack order: a on top so its release
        # frees space for the P2 SBUF pools while b stays live for tch1)
        pool_b = tc.alloc_tile_pool(name="p1b", bufs=1)
        xt_hi = pool_b.tile([128, 8, ND, 128], F16)
        wq_sb = pool_b.tile([128, ND, E], F16)
        wk_sb = pool_b.tile([128, ND, E], F16)
        pool_a = tc.alloc_tile_pool(name="p1a", bufs=1)
        xt_lo = pool_a.tile([128, 8, ND, 128], F16)
        wv_sb = pool_a.tile([128, ND, E], F16)

        def xt_blk(tb):
            return (xt_lo[:, tb] if tb < 8 else xt_hi[:, tb - 8])

        # ---- DMA schedule: wv chunks + first token blocks first so the
        # v-pass first matmul gates on ~640KB; the rest streams under the
        # v-pass compute.
        nc.vector.memset(ones_sb, 1.0)
        # warm the ACT exp table set without waiting on any DMA
        nc.scalar.activation(scratch[0:1, 0:1], ones_sb[0:1, 0:1], EXP)

        nc.sync.dma_start(out=wv_sb[:, 0:4, :], in_=wv.ap()[:, 0:4, :])
        for qq in range(4):
            nc.sync.dma_start(out=xt_lo[:, 0, 4 * qq:4 * qq + 4, :],
                              in_=xt.ap()[:, 0, 4 * qq:4 * qq + 4, :])
        for qq in range(1, 4):
            nc.sync.dma_start(out=wv_sb[:, 4 * qq:4 * qq + 4, :],
                              in_=wv.ap()[:, 4 * qq:4 * qq + 4, :])
        for tb in range(1, 4):
            nc.sync.dma_start(out=xt_lo[:, tb], in_=xt.ap()[:, tb])
        for tb in range(4, 8, 2):
            nc.sync.dma_start(out=xt_lo[:, tb:tb + 2], in_=xt.ap()[:, tb:tb + 2])
        for tb in range(8, 16, 2):
            nc.sync.dma_start(out=xt_hi[:, tb - 8:tb - 6],
                              in_=xt.ap()[:, tb:tb + 2])
        nc.sync.dma_start(out=wq_sb, in_=wq.ap())
        nc.sync.dma_start(out=wk_sb, in_=wk.ap())
        if causal:
            nc.sync.dma_start(out=trip_sb, in_=trip.ap())
        nc.sync.dma_start(out=wo_sb, in_=wo.ap())

        scope_p1 = nc.named_scope("P1_qkv"); scope_p1.__enter__()
        # ---- P1: v (token-major) first, then q/k cols 0:1024 ------------
        for tbg in range(NT):
            ps = ps_c_pool.tile([128, 512], F32, name="ps_v", tag="c")
            for dt in range(ND):
                nc.tensor.matmul(
                    ps, xt_blk(tbg)[:, dt, :],
                    wv_sb[:, dt, :], start=(dt == 0), stop=(dt == ND - 1))
            if tbg % 2 == 0:
                nc.vector.tensor_copy(v_sb[:, tbg, :], ps)
            else:
                nc.scalar.copy(v_sb[:, tbg, :], ps)

        def qk_unit(tch, ti, m, half):
            w_sb = wq_sb if ti == 0 else wk_sb
            dst = qT if ti == 0 else kT
            tb0 = tch * 8 + half * 4
            tsl = slice(tch * 1024 + half * 512, tch * 1024 + (half + 1) * 512)
            ps = ps_c_pool.tile([128, 512], F32, name="ps_qk", tag="c")
            for dt in range(ND):
                src = xt_lo if tb0 < 8 else xt_hi
                b0 = tb0 if tb0 < 8 else tb0 - 8
                nc.tensor.matmul(ps, w_sb[:, dt, m * 128:(m + 1) * 128],
                                 src[:, b0:b0 + 4, dt, :],
                                 start=(dt == 0), stop=(dt == ND - 1))
            if (m + ti + half) % 2 == 0:
                nc.scalar.copy(dst[m][:, tsl], ps)
            else:
                nc.vector.tensor_copy(dst[m][:, tsl], ps)

        for ti in range(2):
            for m in range(NE):
                for half in range(2):
                    qk_unit(0, ti, m, half)
        pool_a.release()
        scope_p1.__exit__(None, None, None)

        scope_p2 = nc.named_scope("P2_attn"); scope_p2.__enter__()
        # ---- P2 SBUF pools (fit in the space pool_a released) -----------
        p_pt = tc.alloc_tile_pool(name="p2pt", bufs=6)
        p_em = tc.alloc_tile_pool(name="p2em", bufs=3)
        p_bs = tc.alloc_tile_pool(name="p2bs", bufs=3)
        p_z = tc.alloc_tile_pool(name="p2z", bufs=3)
        p_ot = tc.alloc_tile_pool(name="p2ot", bufs=6)

        def p3_unit(tt, nch, act_ok=False):
            tsl = slice(tt * 128, (tt + 1) * 128)
            ps_o = ps_c_pool.tile([128, 512], F32, name="ps_o", tag="c")
            for et in range(NE):
                nc.tensor.matmul(
                    ps_o, ctx[et][:, tsl],
                    wo_sb[:, et, nch * 512:(nch + 1) * 512],
                    start=(et == 0), stop=(et == NE - 1))
            ot = p_ot.tile([128, 512], F16, name="ot")
            # inside the attention window ACT is reserved for the exps (an
            # interleaved copy there delays the exp chain and stalls ctx)
            if act_ok and (tt + nch) % 2 == 0:
                nc.scalar.copy(ot, ps_o)
            else:
                nc.vector.tensor_copy(ot, ps_o)
            nc.sync.dma_start(
                out=out.ap()[tsl, nch * 512:(nch + 1) * 512], in_=ot)

        def attn_chunk(ic, filler):
            """Emit attention for i-chunk ic; call filler() in PE slack.

            Key tiles are processed as pseudo-pairs [(jt, qo, qn) x 2]:
            plane j covers query columns [qo, qo+qn).  Causal diagonal
            tiles get shrinking query windows; their triangle lives in
            the first 128 columns of each window (trip mask).
            """
            isl = slice(ic * 512, (ic + 1) * 512)
            surv = [jt for jt in range(NJ) if cls[jt, ic] != SKIP]
            assert surv, f"i-chunk {ic}: every key block masked"
            first, last = surv[0], surv[-1]
            if causal:
                nodiag = [jt for jt in surv if cls[jt, ic] != MIXED]
                diag = [jt for jt in surv if cls[jt, ic] == MIXED]
                prs = [[(jt, 0, 512) for jt in nodiag[i:i + 2]]
                       for i in range(0, len(nodiag), 2)]
                for kk in range(len(diag) // 2):
                    prs.append([(diag[2 * kk + j], 128 * (2 * kk + j),
                                 512 - 128 * (2 * kk + j)) for j in range(2)])
            else:
                prs = [[(jt, 0, 512) for jt in surv[i:i + 2]]
                       for i in range(0, len(surv), 2)]
            for h in range(HPC):
                cps = ps_c_pool.tile([128, 512], F32, name="ps_c", tag="c")
                zt = p_z.tile([128, 2, 512], F16, name="zt")
                zinit = [False, False]

                def flush(prev):
                    ppt, pjs = prev
                    for j, (jt, qo, qn) in enumerate(pjs):
                        nc.tensor.matmul(
                            cps[:, qo:qo + qn],
                            v_sb[:, jt, h * 128:(h + 1) * 128],
                            ppt[:, j, 0:qn], start=(jt == first),
                            stop=(jt == last), skip_group_check=True)

                prev = None
                for pjs in prs:
                    np_ = len(pjs)
                    mx = max(qn for (_, _, qn) in pjs)
                    isdiag = causal and cls[pjs[0][0], ic] == MIXED
                    ps_s = ps_s_pool.tile([128, 2, 512], F32, name="ps_s")
                    for j, (jt, qo, qn) in enumerate(pjs):
                        nc.tensor.matmul(
                            ps_s[:, j, 0:qn], kT[h][:, jt * 128:(jt + 1) * 128],
                            qT[h][:, ic * 512 + qo:ic * 512 + qo + qn],
                            start=True, stop=True)
                    pt = p_pt.tile([128, 2, 512], F16, name="pt")
                    if np_ == 2:
                        nc.scalar.activation(pt[:, :, 0:mx], ps_s[:, :, 0:mx], EXP)
                    else:
                        nc.scalar.activation(pt[:, 0, 0:mx], ps_s[:, 0, 0:mx], EXP)
                    if isdiag:
                        nc.vector.tensor_mul(
                            pt[:, 0:np_, 0:128], pt[:, 0:np_, 0:128],
                            trip_sb[:, 0:np_, :])
                    elif not causal and any(cls[jt, ic] == MIXED
                                            for (jt, _, _) in pjs):
                        emt = p_em.tile([128, 2, 512], F16, name="emt")
                        for j, (jt, _, _) in enumerate(pjs):
                            if cls[jt, ic] == MIXED:
                                nc.sync.dma_start(
                                    out=emt[:, j, :],
                                    in_=em.ap()[jt * 128:(jt + 1) * 128, isl])
                            else:
                                nc.vector.memset(emt[:, j, :], 1.0)
                        if np_ == 2:
                            nc.vector.tensor_mul(pt, pt, emt)
                        else:
                            nc.vector.tensor_mul(
                                pt[:, 0, :], pt[:, 0, :], emt[:, 0, :])
                    # softmax denominator accumulation into the zt planes
                    uniform = (np_ == 2 and all(qn == 512 for (_, _, qn) in pjs))
                    with nc.allow_low_precision(reason="softmax denom f16"):
                        if uniform and not zinit[0] and not zinit[1]:
                            nc.vector.tensor_copy(zt, pt)
                            zinit[0] = zinit[1] = True
                        elif uniform:
                            nc.vector.tensor_add(zt, zt, pt)
                        else:
                            for j, (jt, qo, qn) in enumerate(pjs):
                                zsl = slice(qo, qo + qn)
                                if not zinit[j]:
                                    zinit[j] = True
                                    if qn == 512:
                                        nc.vector.tensor_copy(
                                            zt[:, j, :], pt[:, j, :])
                                    else:
                                        nc.vector.memset(zt[:, j, :], 0.0)
                                        nc.vector.tensor_add(
                                            zt[:, j, zsl], zt[:, j, zsl],
                                            pt[:, j, 0:qn])
                                else:
                                    nc.vector.tensor_add(
                                        zt[:, j, zsl], zt[:, j, zsl],
                                        pt[:, j, 0:qn])
                    if prev is not None:
                        # fill unit BEFORE the ctx flush: covers the first
                        # exp's latency at each head start
                        filler(h)
                        flush(prev)
                    prev = (pt, pjs)
                flush(prev)
                if not zinit[1]:
                    nc.vector.memset(zt[:, 1, :], 0.0)
                lps = ps_l_pool.tile([128, 512], F32, name="ps_l", tag="l")
                for j in range(2):
                    nc.tensor.matmul(lps, ones_sb, zt[:, j, :],
                                     start=(j == 0), stop=(j == 1))
                bsb = p_bs.tile([128, 512], F32, name="bsb")
                nc.vector.reciprocal_approx_fast(out=bsb, in_=lps)
                nc.vector.tensor_mul(ctx[h][:, isl], cps, bsb)
                filler(h, tail=True)

        # ---- i-chunk 0 interleaved with q/k cols 1024:2048 --------------
        # chunk 0 has no P3 filler units, so its exp warmup would stall the
        # PE; instead its pair-steps ride between tch=1 projection units.
        if causal:
            units1 = [(ti, m, half)
                      for ti in range(2) for m in range(NE) for half in range(2)]

            def p1_filler(h, tail=False):
                n = len(units1) if (tail and h == HPC - 1) else 1
                for _ in range(n):
                    if units1:
                        qk_unit(1, *units1.pop(0))
            attn_chunk(0, p1_filler)
            while units1:
                qk_unit(1, *units1.pop(0))
            ic_start = 1
        else:
            for ti in range(2):
                for m in range(NE):
                    for half in range(2):
                        qk_unit(1, ti, m, half)
            ic_start = 0

        # ---- P2 main: chunks with P3 units of chunk ic-1 as filler ------
        for ic in range(ic_start, NI):
            ic_units = [(4 * (ic - 1) + tt, u)
                        for tt in range(4) for u in range(4)] if ic else []
            state = {"h": -1, "skip": 0, "inunits": 0}

            def p3_filler(h, tail=False, ic=ic, ic_units=ic_units, state=state):
                if state["h"] != h:
                    # h0's first slot stays empty: the previous chunk's last
                    # ctx normalization is still draining on the DVE
                    state["h"] = h
                    state["skip"] = 1 if h == 0 else 0
                    state["inunits"] = 0
                if tail:
                    nflush = (len(ic_units) if h == HPC - 1
                              else min(2, len(ic_units)))
                    aok = ic == NI - 1 and h == HPC - 1
                    for _ in range(nflush):
                        p3_unit(*ic_units.pop(0), act_ok=aok)
                    return
                if state["skip"]:
                    state["skip"] -= 1
                elif ic_units and state["inunits"] < 2:
                    p3_unit(*ic_units.pop(0))
                    state["inunits"] += 1
            attn_chunk(ic, p3_filler)
        scope_p2.__exit__(None, None, None)
        scope_p3 = nc.named_scope("P3_out"); scope_p3.__enter__()
        # ---- P3 tail: last token chunk ----------------------------------
        for tt in range(12, 16):
            for nch in range(NI):
                p3_unit(tt, nch, act_ok=True)
        for p in (p_ot, p_z, p_bs, p_em, p_pt):
            p.release()
        pool_b.release()
        for p in (ps_l_pool, ps_c_pool, ps_s_pool):
            p.release()
        pool_qk.release()
        pool_res.release()
        scope_p3.__exit__(None, None, None)

    nc.compile()
    return nc


def _get_nc(cls_key, causal):
    key = (cls_key, causal)
    if key not in _NC_CACHE:
        _NC_CACHE[key] = _build(cls_key, causal)
    return _NC_CACHE[key]


def _causal_pattern(o):
    p = np.arange(128)[:, None]
    f = np.arange(512)[None, :]
    return (p + o * 128 <= f).astype(np.float16)


def kernel(x, Wq, Wk, Wv, Wo, attn_mask):
    x = np.asarray(x, dtype=np.float32)
    Wq = np.asarray(Wq, dtype=np.float32)
    Wk = np.asarray(Wk, dtype=np.float32)
    Wv = np.asarray(Wv, dtype=np.float32)
    Wo = np.asarray(Wo, dtype=np.float32)
    mask = np.asarray(attn_mask, dtype=np.float32).reshape(T, T)

    emT = np.ascontiguousarray(np.exp(mask).T)
    scale = np.float32(1.0 / np.sqrt(DH))

    blocks = emT.reshape(NJ, 128, NI, 512)
    cls = np.full((NJ, NI), MIXED, dtype=np.int64)
    for jt in range(NJ):
        for ic in range(NI):
            sub = blocks[jt, :, ic, :]
            if not sub.any():
                cls[jt, ic] = SKIP
            elif np.all(sub == 1.0):
                cls[jt, ic] = NOMULT
    cls_key = tuple(cls.flatten().tolist())

    # causal fast path: survivors are a prefix, MIXED blocks are the last 4
    # of each i-chunk and match the canonical diagonal patterns
    causal = True
    pat = [_causal_pattern(o).astype(np.float32) for o in range(4)]
    for ic in range(NI):
        surv = [jt for jt in range(NJ) if cls[jt, ic] != SKIP]
        mix = [jt for jt in range(NJ) if cls[jt, ic] == MIXED]
        if surv != list(range(4 * ic + 4)) or mix != list(range(4 * ic, 4 * ic + 4)):
            causal = False
            break
        for jt in mix:
            if not np.array_equal(blocks[jt, :, ic, :], pat[jt - 4 * ic]):
                causal = False
                break
        if not causal:
            break

    # upper-triangle [128,128] pattern, duplicated for both pair planes
    tri = (np.arange(128)[:, None] <= np.arange(128)[None, :])
    trip = np.ascontiguousarray(
        np.broadcast_to(tri.astype(np.float16), (2, 128, 128)).transpose(1, 0, 2))

    def _perm(a, ntile):
        # [ntile*128, F] -> [128, ntile, F] contiguous fp16
        f = a.shape[1]
        return np.ascontiguousarray(
            a.reshape(ntile, 128, f).transpose(1, 0, 2)).astype(np.float16)

    def _perm_blk(a):
        # x[b]: [T, D] -> xt[p, tb, nd, c] = x[tb*128+c, nd*128+p]
        r = a.reshape(NT, 128, ND, 128)
        return np.ascontiguousarray(r.transpose(3, 0, 2, 1)).astype(np.float16)

    xT = [_perm_blk(x[b]) for b in range(B)]
    emT16 = emT.astype(np.float16)

    in_maps = []
    for c in range(8):
        b, g = c // 4, c % 4
        rows = slice(E * g, E * (g + 1))
        m = {
            "xt": xT[b],
            "wq": _perm((Wq[rows, :] * scale).T, ND),
            "wk": _perm(Wk[rows, :].T, ND),
            "wv": _perm(Wv[rows, :].T, ND),
            "wo": _perm(Wo[:, rows].T, NE),
        }
        if causal:
            m["trip"] = trip
        else:
            m["em"] = emT16
        in_maps.append(m)

    global _LAST_IN_MAPS, _LAST_NC
    _LAST_IN_MAPS = in_maps
    nc = _get_nc(cls_key, causal)
    _LAST_NC = nc
    res = run_bass_kernel_spmd(nc, in_maps, list(range(8)))
    outs = [r["out"].astype(np.float32) for r in res.results]
    full = np.stack([
        outs[0] + outs[1] + outs[2] + outs[3],
        outs[4] + outs[5] + outs[6] + outs[7],
    ]).astype(np.float32)
    return full
